# revision 13
# baseline (speedup 1.0000x reference)
"""Multi-head attention (B=2, S=2048, D=1024, H=16, hd=64) on 8 TRN2 cores.

Sharding: core c = (b, hg) with b = c // 4 (batch), hg = c % 4 (head group of
4 heads = 256 hidden features). Each core:
  - projects its batch's q/k/v against its 256-column slice of Wq/Wk/Wv
  - runs attention for its 4 heads (scores kept transposed: [k, q])
  - multiplies its ctx slice against its 256-row slice of Wo.T
The 4 per-batch partial outputs are summed on the host (out_proj all-reduce).

All matmuls run as float32r (fp32 data, full-rate PE mode); softmax skips the
max-subtraction (scores/8 ~ N(0,1), exp never overflows in fp32) so the row
sum can ride along as a 65th output row of the ctx matmul (ones column).
"""

import numpy as np

import concourse.bass as bass
import concourse.bacc as bacc
import concourse.mybir as mybir
import concourse.tile as tile
from concourse import bass_utils

B = 2
S = 2048
DIM = 1024
NH = 16
HD = 64
NCORES = 8
HPC = 4          # heads per core
F = HPC * HD     # 256 features per core
KO = DIM // 128  # 8 contraction chunks for projections
SC = S // 512    # 4 sequence chunks of 512
KT = S // 128    # 16 key tiles of 128
QT = S // 128    # 16 query tiles of 128

MM_DT = mybir.dt.float32r

_CACHE = {}


def _build():
    if "nc" in _CACHE:
        return _CACHE["nc"]

    nc = bacc.Bacc("TRN2", target_bir_lowering=False, debug=False,
                   num_devices=NCORES)

    f32 = mybir.dt.float32
    fr = MM_DT
    xqT = nc.dram_tensor("xqT", [DIM, S], fr, kind="ExternalInput").ap()
    xkT = nc.dram_tensor("xkT", [DIM, S], fr, kind="ExternalInput").ap()
    xvT = nc.dram_tensor("xvT", [DIM, S], fr, kind="ExternalInput").ap()
    wq = nc.dram_tensor("wq", [128, KO, F], fr, kind="ExternalInput").ap()
    wk = nc.dram_tensor("wk", [128, KO, F], fr, kind="ExternalInput").ap()
    wv = nc.dram_tensor("wv", [128, KO, F], fr, kind="ExternalInput").ap()
    wo = nc.dram_tensor("wo", [128, 2, DIM], fr, kind="ExternalInput").ap()
    y = nc.dram_tensor("y", [S, DIM], f32, kind="ExternalOutput").ap()

    # [ki, ko, n] views of the transposed activations
    xqTr = xqT.rearrange("(ko ki) n -> ki ko n", ki=128)
    xkTr = xkT.rearrange("(ko ki) n -> ki ko n", ki=128)
    xvTr = xvT.rearrange("(ko ki) n -> ki ko n", ki=128)

    lowp = nc.allow_low_precision(reason="float32r matmul operand storage")
    lowp.__enter__()
    with tile.TileContext(nc) as tc:
        with (
            tc.tile_pool(name="persist", bufs=1) as persist,
            tc.tile_pool(name="xslab", bufs=2) as xslab,
            tc.tile_pool(name="vslab", bufs=2) as vslab,
            tc.tile_pool(name="expp", bufs=4) as expp,
            tc.tile_pool(name="normp", bufs=2) as normp,
            tc.tile_pool(name="outsb", bufs=3) as outsb,
            tc.tile_pool(name="psA", bufs=4, space="PSUM") as psA,
            tc.tile_pool(name="psC", bufs=3, space="PSUM") as psC,
        ):
            wq_s = persist.tile([128, KO, F], fr, tag="wq")
            wk_s = persist.tile([128, KO, F], fr, tag="wk")
            wv_s = persist.tile([128, KO, F], fr, tag="wv")
            wo_s = persist.tile([128, 2, DIM], fr, tag="wo")
            khT = persist.tile([128, 2, S], fr, tag="khT")
            vh_ext = persist.tile([128, KT, HPC, HD + 1], fr, tag="vh")
            ones_t = persist.tile([65, HD], fr, tag="ones")
            qhT = [persist.tile([128, 2, 512], fr, tag=f"qhT{i}",
                                name=f"qhT{i}") for i in range(SC)]
            ctxT = [persist.tile([128, 2, 512], fr, tag=f"ctxT{i}",
                                 name=f"ctxT{i}") for i in range(SC)]

            nc.sync.dma_start(out=wq_s, in_=wq)
            nc.sync.dma_start(out=wk_s, in_=wk)
            nc.sync.dma_start(out=wv_s, in_=wv)
            nc.sync.dma_start(out=wo_s, in_=wo)
            ones32 = persist.tile([128, HD], f32, tag="ones32")
            nc.vector.memset(ones32[:], 1.0)
            nc.vector.tensor_copy(out=ones_t[:], in_=ones32[0:65, :])
            for kt in range(KT):
                nc.vector.tensor_copy(
                    out=vh_ext[:, kt, :, HD:HD + 1],
                    in_=ones32[:, 0:HPC].unsqueeze(2))

            # ---- k/v projections (full sequence needed before attention) ----
            for sc in range(SC):
                slab = xslab.tile([128, KO, 512], fr, tag="x")
                nc.sync.dma_start(out=slab, in_=xkTr[:, :, sc * 512:(sc + 1) * 512])
                for f in range(2):
                    ps = psA.tile([128, 512], f32, tag="mm")
                    for ko in range(KO):
                        nc.tensor.matmul(
                            ps[:], (wk_s[:, ko, f * 128:(f + 1) * 128]),
                            (slab[:, ko, :]),
                            start=(ko == 0), stop=(ko == KO - 1))
                    nc.vector.tensor_copy(
                        out=khT[:, f, sc * 512:(sc + 1) * 512], in_=ps[:])

            for kt in range(KT):
                vsl = vslab.tile([128, KO, 128], fr, tag="v")
                nc.sync.dma_start(out=vsl, in_=xvTr[:, :, kt * 128:(kt + 1) * 128])
                ps = psA.tile([128, 512], f32, tag="mm")
                for ko in range(KO):
                    nc.tensor.matmul(
                        ps[:, 0:F], (vsl[:, ko, :]), (wv_s[:, ko, :]),
                        start=(ko == 0), stop=(ko == KO - 1))
                nc.vector.tensor_copy(
                    out=vh_ext[:, kt, :, 0:HD],
                    in_=ps[:, 0:F].rearrange("p (h d) -> p h d", h=HPC))

            # ---- per q-chunk: q projection, attention, out projection ----
            for qc in range(SC):
                slab = xslab.tile([128, KO, 512], fr, tag="x")
                nc.sync.dma_start(out=slab, in_=xqTr[:, :, qc * 512:(qc + 1) * 512])
                for f in range(2):
                    ps = psA.tile([128, 512], f32, tag="mm")
                    for ko in range(KO):
                        nc.tensor.matmul(
                            ps[:], (wq_s[:, ko, f * 128:(f + 1) * 128]),
                            (slab[:, ko, :]),
                            start=(ko == 0), stop=(ko == KO - 1))
                    nc.vector.tensor_copy(out=qhT[qc][:, f, :], in_=ps[:])

                for hp in range(2):
                    ps_ctx = [psC.tile([65, 512], f32, tag="ctx",
                                       name=f"ctx{qc}_{hp}_{j}")
                              for j in range(2)]
                    for kt in range(KT):
                        kts = slice(kt * 128, (kt + 1) * 128)
                        for j in range(2):
                            lo, hi = j * 64, (j + 1) * 64
                            ps_s = psA.tile([128, 512], f32, tag="mm")
                            nc.tensor.matmul(
                                ps_s[:], (khT[lo:hi, hp, kts]),
                                (qhT[qc][lo:hi, hp, :]),
                                start=True, stop=True)
                            e = expp.tile([128, 512], fr, tag="e")
                            nc.scalar.activation(
                                out=e[:], in_=ps_s[:],
                                func=mybir.ActivationFunctionType.Exp,
                                scale=0.125)
                            nc.tensor.matmul(
                                ps_ctx[j][:], (vh_ext[:, kt, 2 * hp + j, :]),
                                (e[:]),
                                start=(kt == 0), stop=(kt == KT - 1))
                    for j in range(2):
                        recip = normp.tile([65, 512], fr, tag="recip")
                        nc.vector.reciprocal(
                            out=recip[64:65, :], in_=ps_ctx[j][64:65, :])
                        ps_b = psA.tile([64, 512], f32, tag="mm")
                        nc.tensor.matmul(
                            ps_b[:], (ones_t[64:65, :]), (recip[64:65, :]),
                            start=True, stop=True)
                        bcast = normp.tile([64, 512], f32, tag="bcast")
                        nc.vector.tensor_copy(out=bcast[:], in_=ps_b[:])
                        if j == 0:
                            nc.vector.tensor_mul(
                                ctxT[qc][0:64, hp, :], ps_ctx[j][0:64, :],
                                bcast[:])
                        else:
                            stage = normp.tile([64, 512], fr, tag="stage")
                            nc.vector.tensor_mul(
                                stage[:], ps_ctx[j][0:64, :], bcast[:])
                            nc.sync.dma_start(
                                out=ctxT[qc][64:128, hp, :], in_=stage[:])

                for qt in range(4):
                    qts = slice(qt * 128, (qt + 1) * 128)
                    ysb = outsb.tile([128, DIM], f32, tag="y")
                    for oc in range(2):
                        ps = psA.tile([128, 512], f32, tag="mm")
                        for fc in range(2):
                            nc.tensor.matmul(
                                ps[:], (ctxT[qc][:, fc, qts]),
                                (wo_s[:, fc, oc * 512:(oc + 1) * 512]),
                                start=(fc == 0), stop=(fc == 1))
                        nc.vector.tensor_copy(
                            out=ysb[:, oc * 512:(oc + 1) * 512], in_=ps[:])
                    nc.sync.dma_start(
                        out=y[qc * 512 + qt * 128:qc * 512 + (qt + 1) * 128, :],
                        in_=ysb[:])

    lowp.__exit__(None, None, None)
    nc.finalize()
    _CACHE["nc"] = nc
    return nc


def _prep_inputs(q, k, v, Wq, Wk, Wv, Wo):
    q = np.ascontiguousarray(np.asarray(q, dtype=np.float32))
    k = np.ascontiguousarray(np.asarray(k, dtype=np.float32))
    v = np.ascontiguousarray(np.asarray(v, dtype=np.float32))
    Wq = np.asarray(Wq, dtype=np.float32)
    Wk = np.asarray(Wk, dtype=np.float32)
    Wv = np.asarray(Wv, dtype=np.float32)
    Wo = np.asarray(Wo, dtype=np.float32)

    xT = {b: {n: np.ascontiguousarray(x[b].T)
              for n, x in (("q", q), ("k", k), ("v", v))}
          for b in range(B)}

    def wslice(W, hg):
        # [128, KO, F] view of Wx.T[:, hg*F:(hg+1)*F]
        s = np.ascontiguousarray(W.T[:, hg * F:(hg + 1) * F]
                                 .reshape(KO, 128, F).transpose(1, 0, 2))
        return s

    def woslice(Wo, hg):
        s = np.ascontiguousarray(Wo.T[hg * F:(hg + 1) * F, :]
                                 .reshape(2, 128, DIM).transpose(1, 0, 2))
        return s

    in_maps = []
    for c in range(NCORES):
        b, hg = c // 4, c % 4
        in_maps.append({
            "xqT": xT[b]["q"], "xkT": xT[b]["k"], "xvT": xT[b]["v"],
            "wq": wslice(Wq, hg), "wk": wslice(Wk, hg), "wv": wslice(Wv, hg),
            "wo": woslice(Wo, hg),
        })
    return in_maps


def kernel(q, k, v, Wq, Wk, Wv, Wo, _trace=False):
    nc = _build()
    in_maps = _prep_inputs(q, k, v, Wq, Wk, Wv, Wo)
    res = bass_utils.run_bass_kernel_spmd(
        nc, in_maps, core_ids=list(range(NCORES)), trace=_trace)
    out = np.zeros((B, S, DIM), dtype=np.float32)
    for c in range(NCORES):
        out[c // 4] += res.results[c]["y"]
    if _trace:
        return out, res
    return out


# revision 14
# speedup vs baseline: 1.0629x; 1.0629x over previous
"""Multi-head attention (B=2, S=2048, D=1024, H=16, hd=64) on 8 TRN2 cores.

Sharding: core c = (b, hg) with b = c // 4 (batch), hg = c % 4 (head group of
4 heads = 256 hidden features). Each core:
  - projects its batch's q/k/v against its 256-column slice of Wq/Wk/Wv
  - runs attention for its 4 heads (scores kept transposed: [k, q])
  - multiplies its ctx slice against its 256-row slice of Wo.T
The 4 per-batch partial outputs are summed on the host (out_proj all-reduce).

All matmuls run as float32r (fp32 data, full-rate PE mode); softmax skips the
max-subtraction (scores/8 ~ N(0,1), exp never overflows in fp32) so the row
sum can ride along as a 65th output row of the ctx matmul (ones column).
"""

import numpy as np

import concourse.bass as bass
import concourse.bacc as bacc
import concourse.mybir as mybir
import concourse.tile as tile
from concourse import bass_utils

B = 2
S = 2048
DIM = 1024
NH = 16
HD = 64
NCORES = 8
HPC = 4          # heads per core
F = HPC * HD     # 256 features per core
KO = DIM // 128  # 8 contraction chunks for projections
SC = S // 512    # 4 sequence chunks of 512
KT = S // 128    # 16 key tiles of 128
QT = S // 128    # 16 query tiles of 128

MM_DT = mybir.dt.float32r

_CACHE = {}


def _build():
    if "nc" in _CACHE:
        return _CACHE["nc"]

    nc = bacc.Bacc("TRN2", target_bir_lowering=False, debug=False,
                   num_devices=NCORES)

    f32 = mybir.dt.float32
    fr = MM_DT
    xqT = nc.dram_tensor("xqT", [DIM, S], fr, kind="ExternalInput").ap()
    xkT = nc.dram_tensor("xkT", [DIM, S], fr, kind="ExternalInput").ap()
    xvT = nc.dram_tensor("xvT", [DIM, S], fr, kind="ExternalInput").ap()
    wq = nc.dram_tensor("wq", [128, KO, F], fr, kind="ExternalInput").ap()
    wk = nc.dram_tensor("wk", [128, KO, F], fr, kind="ExternalInput").ap()
    wv = nc.dram_tensor("wv", [128, KO, F], fr, kind="ExternalInput").ap()
    wo = nc.dram_tensor("wo", [128, 2, DIM], fr, kind="ExternalInput").ap()
    y = nc.dram_tensor("y", [S, DIM], f32, kind="ExternalOutput").ap()

    # [ki, ko, n] views of the transposed activations
    xqTr = xqT.rearrange("(ko ki) n -> ki ko n", ki=128)
    xkTr = xkT.rearrange("(ko ki) n -> ki ko n", ki=128)
    xvTr = xvT.rearrange("(ko ki) n -> ki ko n", ki=128)

    lowp = nc.allow_low_precision(reason="float32r matmul operand storage")
    lowp.__enter__()
    with tile.TileContext(nc) as tc:
        with (
            tc.tile_pool(name="persist", bufs=1) as persist,
            tc.tile_pool(name="xslab", bufs=2) as xslab,
            tc.tile_pool(name="expp", bufs=4) as expp,
            tc.tile_pool(name="normp", bufs=2) as normp,
            tc.tile_pool(name="outsb", bufs=3) as outsb,
            tc.tile_pool(name="psA", bufs=4, space="PSUM") as psA,
            tc.tile_pool(name="psC", bufs=4, space="PSUM") as psC,
        ):
            wq_s = persist.tile([128, KO, F], fr, tag="wq")
            wk_s = persist.tile([128, KO, F], fr, tag="wk")
            wv_s = persist.tile([128, KO, F], fr, tag="wv")
            wo_s = persist.tile([128, 2, DIM], fr, tag="wo")
            khT = persist.tile([128, 2, S], fr, tag="khT")
            vh_ext = persist.tile([128, KT, HPC, HD + 1], fr, tag="vh")
            ones_t = persist.tile([65, HD], fr, tag="ones")
            qhT = [persist.tile([128, 2, 512], fr, tag=f"qhT{i}",
                                name=f"qhT{i}") for i in range(SC)]
            ctxT = [persist.tile([128, 2, 512], fr, tag=f"ctxT{i}",
                                 name=f"ctxT{i}") for i in range(SC)]

            nc.sync.dma_start(out=wk_s, in_=wk)
            nc.sync.dma_start(out=wv_s, in_=wv)
            nc.sync.dma_start(out=wq_s, in_=wq)
            nc.sync.dma_start(out=wo_s, in_=wo)
            ones32 = persist.tile([128, HD], f32, tag="ones32")
            nc.vector.memset(ones32[:], 1.0)
            nc.vector.tensor_copy(out=ones_t[:], in_=ones32[0:65, :])
            for kt in range(KT):
                nc.vector.tensor_copy(
                    out=vh_ext[:, kt, :, HD:HD + 1],
                    in_=ones32[:, 0:HPC].unsqueeze(2))

            # ---- k/v projections (full sequence needed before attention) ----
            for sc in range(SC):
                kslab = xslab.tile([128, KO, 512], fr, tag="x", name=f"kslab{sc}")
                nc.sync.dma_start(out=kslab, in_=xkTr[:, :, sc * 512:(sc + 1) * 512])
                for f in range(2):
                    ps = psA.tile([128, 512], f32, tag="mm")
                    for ko in range(KO):
                        nc.tensor.matmul(
                            ps[:], (wk_s[:, ko, f * 128:(f + 1) * 128]),
                            (kslab[:, ko, :]),
                            start=(ko == 0), stop=(ko == KO - 1))
                    nc.vector.tensor_copy(
                        out=khT[:, f, sc * 512:(sc + 1) * 512], in_=ps[:])
                vslb = xslab.tile([128, KO, 512], fr, tag="x", name=f"vslab{sc}")
                nc.sync.dma_start(out=vslb, in_=xvTr[:, :, sc * 512:(sc + 1) * 512])
                for kt4 in range(4):
                    kt = sc * 4 + kt4
                    ps = psA.tile([128, 512], f32, tag="mm")
                    for ko in range(KO):
                        nc.tensor.matmul(
                            ps[:, 0:F],
                            (vslb[:, ko, kt4 * 128:(kt4 + 1) * 128]),
                            (wv_s[:, ko, :]),
                            start=(ko == 0), stop=(ko == KO - 1))
                    nc.vector.tensor_copy(
                        out=vh_ext[:, kt, :, 0:HD],
                        in_=ps[:, 0:F].rearrange("p (h d) -> p h d", h=HPC))

            # ---- per q-chunk: q projection, attention, out projection ----
            for qc in range(SC):
                slab = xslab.tile([128, KO, 512], fr, tag="x")
                nc.sync.dma_start(out=slab, in_=xqTr[:, :, qc * 512:(qc + 1) * 512])
                for f in range(2):
                    ps = psA.tile([128, 512], f32, tag="mm")
                    for ko in range(KO):
                        nc.tensor.matmul(
                            ps[:], (wq_s[:, ko, f * 128:(f + 1) * 128]),
                            (slab[:, ko, :]),
                            start=(ko == 0), stop=(ko == KO - 1))
                    nc.vector.tensor_copy(out=qhT[qc][:, f, :], in_=ps[:])

                for hp in range(2):
                    ps_ctx = [psC.tile([65, 512], f32, tag="ctx",
                                       name=f"ctx{qc}_{hp}_{j}")
                              for j in range(2)]
                    for kt in range(KT):
                        kts = slice(kt * 128, (kt + 1) * 128)
                        for j in range(2):
                            lo, hi = j * 64, (j + 1) * 64
                            ps_s = psA.tile([128, 512], f32, tag="mm")
                            nc.tensor.matmul(
                                ps_s[:], (khT[lo:hi, hp, kts]),
                                (qhT[qc][lo:hi, hp, :]),
                                start=True, stop=True)
                            e = expp.tile([128, 512], fr, tag="e")
                            nc.scalar.activation(
                                out=e[:], in_=ps_s[:],
                                func=mybir.ActivationFunctionType.Exp,
                                scale=0.125)
                            nc.tensor.matmul(
                                ps_ctx[j][:], (vh_ext[:, kt, 2 * hp + j, :]),
                                (e[:]),
                                start=(kt == 0), stop=(kt == KT - 1))
                    for j in range(2):
                        sums = normp.tile([65, 512], fr, tag="sums")
                        nc.scalar.copy(
                            out=sums[64:65, :], in_=ps_ctx[j][64:65, :])
                        ps_b = psA.tile([64, 512], f32, tag="mm")
                        nc.tensor.matmul(
                            ps_b[:], (ones_t[64:65, :]), (sums[64:65, :]),
                            start=True, stop=True)
                        rec = normp.tile([64, 512], f32, tag="rec")
                        nc.vector.reciprocal(out=rec[:], in_=ps_b[:])
                        if j == 0:
                            nc.vector.tensor_mul(
                                ctxT[qc][0:64, hp, :], ps_ctx[j][0:64, :],
                                rec[:])
                        else:
                            stage = normp.tile([64, 512], fr, tag="stage")
                            nc.vector.tensor_mul(
                                stage[:], ps_ctx[j][0:64, :], rec[:])
                            nc.sync.dma_start(
                                out=ctxT[qc][64:128, hp, :], in_=stage[:])

                for qt in range(4):
                    qts = slice(qt * 128, (qt + 1) * 128)
                    ysb = outsb.tile([128, DIM], f32, tag="y")
                    for oc in range(2):
                        ps = psA.tile([128, 512], f32, tag="mm")
                        for fc in range(2):
                            nc.tensor.matmul(
                                ps[:], (ctxT[qc][:, fc, qts]),
                                (wo_s[:, fc, oc * 512:(oc + 1) * 512]),
                                start=(fc == 0), stop=(fc == 1))
                        nc.vector.tensor_copy(
                            out=ysb[:, oc * 512:(oc + 1) * 512], in_=ps[:])
                    nc.sync.dma_start(
                        out=y[qc * 512 + qt * 128:qc * 512 + (qt + 1) * 128, :],
                        in_=ysb[:])

    lowp.__exit__(None, None, None)
    nc.finalize()
    _CACHE["nc"] = nc
    return nc


def _prep_inputs(q, k, v, Wq, Wk, Wv, Wo):
    q = np.ascontiguousarray(np.asarray(q, dtype=np.float32))
    k = np.ascontiguousarray(np.asarray(k, dtype=np.float32))
    v = np.ascontiguousarray(np.asarray(v, dtype=np.float32))
    Wq = np.asarray(Wq, dtype=np.float32)
    Wk = np.asarray(Wk, dtype=np.float32)
    Wv = np.asarray(Wv, dtype=np.float32)
    Wo = np.asarray(Wo, dtype=np.float32)

    xT = {b: {n: np.ascontiguousarray(x[b].T)
              for n, x in (("q", q), ("k", k), ("v", v))}
          for b in range(B)}

    def wslice(W, hg):
        # [128, KO, F] view of Wx.T[:, hg*F:(hg+1)*F]
        s = np.ascontiguousarray(W.T[:, hg * F:(hg + 1) * F]
                                 .reshape(KO, 128, F).transpose(1, 0, 2))
        return s

    def woslice(Wo, hg):
        s = np.ascontiguousarray(Wo.T[hg * F:(hg + 1) * F, :]
                                 .reshape(2, 128, DIM).transpose(1, 0, 2))
        return s

    in_maps = []
    for c in range(NCORES):
        b, hg = c // 4, c % 4
        in_maps.append({
            "xqT": xT[b]["q"], "xkT": xT[b]["k"], "xvT": xT[b]["v"],
            "wq": wslice(Wq, hg), "wk": wslice(Wk, hg), "wv": wslice(Wv, hg),
            "wo": woslice(Wo, hg),
        })
    return in_maps


def kernel(q, k, v, Wq, Wk, Wv, Wo, _trace=False):
    nc = _build()
    in_maps = _prep_inputs(q, k, v, Wq, Wk, Wv, Wo)
    res = bass_utils.run_bass_kernel_spmd(
        nc, in_maps, core_ids=list(range(NCORES)), trace=_trace)
    out = np.zeros((B, S, DIM), dtype=np.float32)
    for c in range(NCORES):
        out[c // 4] += res.results[c]["y"]
    if _trace:
        return out, res
    return out


# revision 17
# speedup vs baseline: 1.4354x; 1.3505x over previous
"""Multi-head attention (B=2, S=2048, D=1024, H=16, hd=64) on 8 TRN2 cores.

Sharding: core c = (b, hg) with b = c // 4 (batch), hg = c % 4 (head group of
4 heads = 256 hidden features). Each core:
  - projects its batch's q/k/v against its 256-column slice of Wq/Wk/Wv
  - runs attention for its 4 heads (scores kept transposed: [k, q])
  - multiplies its ctx slice against its 256-row slice of Wo.T
The 4 per-batch partial outputs are summed on the host (out_proj all-reduce).

All matmuls run as float32r (fp32 data, full-rate PE mode); softmax skips the
max-subtraction (scores/8 ~ N(0,1), exp never overflows in fp32) so the row
sum can ride along as a 65th output row of the ctx matmul (ones column).
"""

import numpy as np

import concourse.bass as bass
import concourse.bacc as bacc
import concourse.mybir as mybir
import concourse.tile as tile
from concourse import bass_utils

B = 2
S = 2048
DIM = 1024
NH = 16
HD = 64
NCORES = 8
HPC = 4          # heads per core
F = HPC * HD     # 256 features per core
KO = DIM // 128  # 8 contraction chunks for projections
SC = S // 512    # 4 sequence chunks of 512
KT = S // 128    # 16 key tiles of 128
QT = S // 128    # 16 query tiles of 128

MM_DT = mybir.dt.float32r

_CACHE = {}


def _build():
    if "nc" in _CACHE:
        return _CACHE["nc"]

    nc = bacc.Bacc("TRN2", target_bir_lowering=False, debug=False,
                   num_devices=NCORES)

    f32 = mybir.dt.float32
    fr = MM_DT
    xqT = nc.dram_tensor("xqT", [DIM, S], fr, kind="ExternalInput").ap()
    xkT = nc.dram_tensor("xkT", [DIM, S], fr, kind="ExternalInput").ap()
    xvT = nc.dram_tensor("xvT", [DIM, S], fr, kind="ExternalInput").ap()
    wq = nc.dram_tensor("wq", [128, KO, F], fr, kind="ExternalInput").ap()
    wk = nc.dram_tensor("wk", [128, KO, F], fr, kind="ExternalInput").ap()
    wv = nc.dram_tensor("wv", [128, KO, F], fr, kind="ExternalInput").ap()
    wo = nc.dram_tensor("wo", [128, 2, DIM], fr, kind="ExternalInput").ap()
    y = nc.dram_tensor("y", [S, DIM], f32, kind="ExternalOutput").ap()

    # [ki, ko, n] views of the transposed activations
    xqTr = xqT.rearrange("(ko ki) n -> ki ko n", ki=128)
    xkTr = xkT.rearrange("(ko ki) n -> ki ko n", ki=128)
    xvTr = xvT.rearrange("(ko ki) n -> ki ko n", ki=128)

    lowp = nc.allow_low_precision(reason="float32r matmul operand storage")
    lowp.__enter__()
    with tile.TileContext(nc) as tc:
        with (
            tc.tile_pool(name="persist", bufs=1) as persist,
            tc.tile_pool(name="xslab", bufs=2) as xslab,
            tc.tile_pool(name="expp", bufs=4) as expp,
            tc.tile_pool(name="normp", bufs=2) as normp,
            tc.tile_pool(name="outsb", bufs=3) as outsb,
            tc.tile_pool(name="psA", bufs=2, space="PSUM") as psA,
            tc.tile_pool(name="psS", bufs=2, space="PSUM") as psS,
            tc.tile_pool(name="psC", bufs=2, space="PSUM") as psC,
        ):
            wq_s = persist.tile([128, KO, F], fr, tag="wq")
            wk_s = persist.tile([128, KO, F], fr, tag="wk")
            wv_s = persist.tile([128, KO, F], fr, tag="wv")
            wo_s = persist.tile([128, 2, DIM], fr, tag="wo")
            khT = persist.tile([128, 2, S], fr, tag="khT")
            vh_ext = persist.tile([128, KT, HPC, HD + 1], fr, tag="vh")
            ones_t = persist.tile([65, HD], fr, tag="ones")
            qhT = [persist.tile([128, 2, 512], fr, tag=f"qhT{i}",
                                name=f"qhT{i}") for i in range(SC)]
            ctxT = [persist.tile([128, 2, 512], fr, tag=f"ctxT{i}",
                                 name=f"ctxT{i}") for i in range(SC)]

            nc.sync.dma_start(out=wk_s, in_=wk)
            nc.sync.dma_start(out=wv_s, in_=wv)
            nc.sync.dma_start(out=wq_s, in_=wq)
            nc.sync.dma_start(out=wo_s, in_=wo)
            ones32 = persist.tile([128, HD], f32, tag="ones32")
            nc.vector.memset(ones32[:], 1.0)
            nc.vector.tensor_copy(out=ones_t[:], in_=ones32[0:65, :])
            for kt in range(KT):
                nc.vector.tensor_copy(
                    out=vh_ext[:, kt, :, HD:HD + 1],
                    in_=ones32[:, 0:HPC].unsqueeze(2))

            # ---- k/v projections (full sequence needed before attention) ----
            for sc in range(SC):
                kslab = xslab.tile([128, KO, 512], fr, tag="x", name=f"kslab{sc}")
                nc.sync.dma_start(out=kslab, in_=xkTr[:, :, sc * 512:(sc + 1) * 512])
                for f in range(2):
                    ps = psA.tile([128, 512], f32, tag="mm")
                    for ko in range(KO):
                        nc.tensor.matmul(
                            ps[:], (wk_s[:, ko, f * 128:(f + 1) * 128]),
                            (kslab[:, ko, :]),
                            start=(ko == 0), stop=(ko == KO - 1))
                    nc.vector.tensor_copy(
                        out=khT[:, f, sc * 512:(sc + 1) * 512], in_=ps[:])
                vslb = xslab.tile([128, KO, 512], fr, tag="x", name=f"vslab{sc}")
                nc.sync.dma_start(out=vslb, in_=xvTr[:, :, sc * 512:(sc + 1) * 512])
                for kt4 in range(4):
                    kt = sc * 4 + kt4
                    ps = psA.tile([128, 512], f32, tag="mm")
                    for ko in range(KO):
                        nc.tensor.matmul(
                            ps[:, 0:F],
                            (vslb[:, ko, kt4 * 128:(kt4 + 1) * 128]),
                            (wv_s[:, ko, :]),
                            start=(ko == 0), stop=(ko == KO - 1))
                    nc.vector.tensor_copy(
                        out=vh_ext[:, kt, :, 0:HD],
                        in_=ps[:, 0:F].rearrange("p (h d) -> p h d", h=HPC))

            # ---- per q-chunk: q projection, attention, out projection ----
            pending = []

            def _norm_tail(qc, hp, j, sums, ctxu):
                def emit():
                    ps_b = psA.tile([128, 512], f32, tag="mm",
                                    name=f"bc{qc}_{hp}_{j}")
                    nc.tensor.matmul(
                        ps_b[0:64, :], (ones_t[64:65, :]), (sums[64:65, :]),
                        start=True, stop=True)
                    rec = normp.tile([64, 512], f32, tag="rec",
                                     name=f"rec{qc}_{hp}_{j}")
                    nc.vector.reciprocal(out=rec[:], in_=ps_b[0:64, :])
                    if j == 0:
                        nc.vector.tensor_mul(
                            ctxT[qc][0:64, hp, :], ctxu[0:64, :], rec[:])
                    else:
                        stage = normp.tile([64, 512], fr, tag="stage",
                                           name=f"st{qc}_{hp}")
                        nc.vector.tensor_mul(stage[:], ctxu[0:64, :], rec[:])
                        nc.sync.dma_start(
                            out=ctxT[qc][64:128, hp, :], in_=stage[:])
                return emit

            for qc in range(SC):
                slab = xslab.tile([128, KO, 512], fr, tag="x")
                nc.sync.dma_start(out=slab, in_=xqTr[:, :, qc * 512:(qc + 1) * 512])
                for f in range(2):
                    ps = psA.tile([128, 512], f32, tag="mm")
                    for ko in range(KO):
                        nc.tensor.matmul(
                            ps[:], (wq_s[:, ko, f * 128:(f + 1) * 128]),
                            (slab[:, ko, :]),
                            start=(ko == 0), stop=(ko == KO - 1))
                    nc.vector.tensor_copy(out=qhT[qc][:, f, :], in_=ps[:])

                for hp in range(2):
                    ps_ctx = [psC.tile([65, 512], f32, tag="ctx",
                                       name=f"ctx{qc}_{hp}_{j}")
                              for j in range(2)]
                    for kt in range(KT):
                        kts = slice(kt * 128, (kt + 1) * 128)
                        ps_s2 = psS.tile([128, 2, 512], f32, tag="s2",
                                         name=f"s2_{qc}_{hp}_{kt}")
                        for j in range(2):
                            lo, hi = j * 64, (j + 1) * 64
                            nc.tensor.matmul(
                                ps_s2[:, j, :], (khT[lo:hi, hp, kts]),
                                (qhT[qc][lo:hi, hp, :]),
                                start=True, stop=True)
                        e2 = expp.tile([128, 2, 512], fr, tag="e")
                        nc.scalar.activation(
                            out=e2[:], in_=ps_s2[:],
                            func=mybir.ActivationFunctionType.Exp,
                            scale=0.125)
                        for j in range(2):
                            nc.tensor.matmul(
                                ps_ctx[j][:], (vh_ext[:, kt, 2 * hp + j, :]),
                                (e2[:, j, :]),
                                start=(kt == 0), stop=(kt == KT - 1))
                        if kt == 2 and pending:
                            pending.pop(0)()
                        elif kt == 5 and pending:
                            pending.pop(0)()
                    # free the ctx PSUM slots fast: copy sums + unnormalized
                    # ctx rows to SBUF; normalization tail is deferred.
                    for j in range(2):
                        lo, hi = j * 64, (j + 1) * 64
                        sums = normp.tile([65, 512], fr, tag="sums",
                                          name=f"sums{qc}_{hp}_{j}")
                        nc.vector.tensor_copy(
                            out=sums[64:65, :], in_=ps_ctx[j][64:65, :])
                        ctxu = normp.tile([128, 512], f32, tag="ctxu",
                                          name=f"ctxu{qc}_{hp}_{j}")
                        nc.vector.tensor_copy(
                            out=ctxu[0:64, :], in_=ps_ctx[j][0:64, :])
                        pending.append(_norm_tail(qc, hp, j, sums, ctxu))

                while pending:
                    pending.pop(0)()
                for qt in range(4):
                    qts = slice(qt * 128, (qt + 1) * 128)
                    ysb = outsb.tile([128, DIM], f32, tag="y")
                    for oc in range(2):
                        ps = psA.tile([128, 512], f32, tag="mm")
                        for fc in range(2):
                            nc.tensor.matmul(
                                ps[:], (ctxT[qc][:, fc, qts]),
                                (wo_s[:, fc, oc * 512:(oc + 1) * 512]),
                                start=(fc == 0), stop=(fc == 1))
                        nc.vector.tensor_copy(
                            out=ysb[:, oc * 512:(oc + 1) * 512], in_=ps[:])
                    nc.sync.dma_start(
                        out=y[qc * 512 + qt * 128:qc * 512 + (qt + 1) * 128, :],
                        in_=ysb[:])

    lowp.__exit__(None, None, None)
    nc.finalize()
    _CACHE["nc"] = nc
    return nc


def _prep_inputs(q, k, v, Wq, Wk, Wv, Wo):
    q = np.ascontiguousarray(np.asarray(q, dtype=np.float32))
    k = np.ascontiguousarray(np.asarray(k, dtype=np.float32))
    v = np.ascontiguousarray(np.asarray(v, dtype=np.float32))
    Wq = np.asarray(Wq, dtype=np.float32)
    Wk = np.asarray(Wk, dtype=np.float32)
    Wv = np.asarray(Wv, dtype=np.float32)
    Wo = np.asarray(Wo, dtype=np.float32)

    xT = {b: {n: np.ascontiguousarray(x[b].T)
              for n, x in (("q", q), ("k", k), ("v", v))}
          for b in range(B)}

    def wslice(W, hg):
        # [128, KO, F] view of Wx.T[:, hg*F:(hg+1)*F]
        s = np.ascontiguousarray(W.T[:, hg * F:(hg + 1) * F]
                                 .reshape(KO, 128, F).transpose(1, 0, 2))
        return s

    def woslice(Wo, hg):
        s = np.ascontiguousarray(Wo.T[hg * F:(hg + 1) * F, :]
                                 .reshape(2, 128, DIM).transpose(1, 0, 2))
        return s

    in_maps = []
    for c in range(NCORES):
        b, hg = c // 4, c % 4
        in_maps.append({
            "xqT": xT[b]["q"], "xkT": xT[b]["k"], "xvT": xT[b]["v"],
            "wq": wslice(Wq, hg), "wk": wslice(Wk, hg), "wv": wslice(Wv, hg),
            "wo": woslice(Wo, hg),
        })
    return in_maps


def kernel(q, k, v, Wq, Wk, Wv, Wo, _trace=False):
    nc = _build()
    in_maps = _prep_inputs(q, k, v, Wq, Wk, Wv, Wo)
    res = bass_utils.run_bass_kernel_spmd(
        nc, in_maps, core_ids=list(range(NCORES)), trace=_trace)
    out = np.zeros((B, S, DIM), dtype=np.float32)
    for c in range(NCORES):
        out[c // 4] += res.results[c]["y"]
    if _trace:
        return out, res
    return out
